# revision 10
# baseline (speedup 1.0000x reference)
"""Centered locally-connected 1x1 conv on 8 TRN2 NeuronCores.

Math (G=1 squeezed):
    out_s[b,j,h,w] = sum_i (x+b)[b,i,h,w] * w[i,j,h,w]
    m[b,j]         = (1/(H*W)) * sum_{i,h,w} b[b,i,h,w] * w[i,j,h,w]
    out            = out_s - m

Sharding: H split across the 8 cores (6 rows each); every (h,w) location is an
independent [CI]x[CI,CO] contraction, so each core reads only its slice of
x/b/weights.  The spatial mean of the b-path needs a cross-core reduction of a
[CO,B] partial sum (16 KB AllReduce).

Per-core device program (288 locations):
  - one fp32 matmul per location: stationary w_l [K=128i, M=128j], moving
    [s_l | b_l] [K=128i, N=64]; PSUM gets [128j, 32 s-out | 32 b-out].
  - s-halves are copied (DVE) into a resident SBUF buffer [128, 288*32],
    b-halves are reduced (DVE) into per-group partial sums.
  - AllReduce the [128,32] b-path sum, scale by 1/(H*W), broadcast, subtract,
    and DMA the result out in chunks.

Inputs for one chunk (48 locations) are packed into a single DRAM region
(w block then [s|b] block) so each chunk is one DMA and the PE's first
matmul of a chunk has a single DMA wait.
"""

import os
from contextlib import ExitStack

import numpy as np

import concourse.bass as bass
import concourse.mybir as mybir
import concourse.tile as tile
from concourse import bacc
from concourse.bass_utils import run_bass_kernel_spmd

B, CI, H, W, CO = 32, 128, 48, 48, 128
NCORES = 8
HL = H // NCORES          # 6 h-rows per core
LOC = HL * W              # 288 locations per core
GRP = 8                   # locations per PSUM bank group
NGRP = LOC // GRP         # 36 groups
CHUNK_L = W               # 48 locations (one h-row) per DMA chunk
NCHUNK = LOC // CHUNK_L   # 6 chunks
WCOLS = CHUNK_L * 128     # 6144 w cols per chunk
SBCOLS = CHUNK_L * 64     # 3072 [s|b] cols per chunk
DCOLS = WCOLS + SBCOLS    # 9216 cols per packed chunk

F32 = mybir.dt.float32

LAST_EXEC_TIME_NS = None
_NC_CACHE = {}


def _build_nc(reps: int = 1):
    nc = bacc.Bacc(None)
    dat_d = nc.declare_dram_parameter("dat", [128, NCHUNK * DCOLS], F32, isOutput=False)
    out_d = nc.declare_dram_parameter("out", [128, LOC * 32], F32, isOutput=True)

    with tile.TileContext(nc) as tc, ExitStack() as ctx:
        dp_in = ctx.enter_context(tc.tile_pool(name="dpin", bufs=3))
        # Two PSUM pools: chunk-first groups draw from a separate pool so
        # their slot-recycle deps are old enough that Tile emits no PE/DVE
        # wait on the chunk's first matmul — it carries only the DMA wait.
        # (walrus allows at most 2 sync waits per LDWEIGHTS/MATMUL.)
        pp = ctx.enter_context(tc.tile_pool(name="pp", bufs=5, space="PSUM"))
        pp0 = ctx.enter_context(tc.tile_pool(name="pp0", bufs=2, space="PSUM"))
        ocp = ctx.enter_context(tc.tile_pool(name="ocp", bufs=NCHUNK + 1))
        sp = ctx.enter_context(tc.tile_pool(name="sp", bufs=2))
        dp = ctx.enter_context(tc.tile_pool(name="dp", bufs=2, space="DRAM"))

        OC = CHUNK_L * 32  # out cols per chunk (1536)

        for r in range(reps):
            oc_ts = []
            bpart_t = sp.tile([128, NGRP * 32], F32, name=f"bp{r}", tag="bp")
            for c in range(NCHUNK):
                dat_t = dp_in.tile([128, DCOLS], F32, name=f"dat{r}_{c}", tag="dat")
                nc.sync.dma_start(dat_t[:], dat_d[:, c * DCOLS : (c + 1) * DCOLS])
                oc_t = ocp.tile([128, OC], F32, name=f"oc{r}_{c}", tag="oc")
                oc_ts.append(oc_t)

                for g in range(CHUNK_L // GRP):
                    pool = pp0 if g == 0 else pp
                    pg = pool.tile(
                        [128, GRP * 64],
                        F32,
                        name=f"pg{r}_{c}_{g}",
                        tag="pg0" if g == 0 else "pg",
                    )
                    for k in range(GRP):
                        l = g * GRP + k  # location within chunk
                        nc.tensor.matmul(
                            pg[:, k * 64 : (k + 1) * 64],
                            lhsT=dat_t[:, l * 128 : (l + 1) * 128],
                            rhs=dat_t[:, WCOLS + l * 64 : WCOLS + (l + 1) * 64],
                            start=True,
                            stop=True,
                        )
                    gi = c * (CHUNK_L // GRP) + g
                    pv = pg[:].rearrange("p (l n) -> p l n", l=GRP)
                    nc.vector.tensor_copy(
                        out=oc_t[:, g * GRP * 32 : (g + 1) * GRP * 32].rearrange(
                            "p (l n) -> p l n", l=GRP
                        ),
                        in_=pv[:, :, 0:32],
                    )
                    pb = pg[:].rearrange("p (l n) -> p n l", l=GRP)[:, 32:64, :]
                    nc.vector.tensor_reduce(
                        out=bpart_t[:, gi * 32 : (gi + 1) * 32],
                        in_=pb,
                        axis=mybir.AxisListType.X,
                        op=mybir.AluOpType.add,
                    )

            # local b-path sum over all groups -> [128, 32]
            bsum_t = sp.tile([128, 32], F32, name=f"bs{r}", tag="bs")
            nc.vector.tensor_reduce(
                out=bsum_t[:],
                in_=bpart_t[:].rearrange("p (g n) -> p n g", g=NGRP),
                axis=mybir.AxisListType.X,
                op=mybir.AluOpType.add,
            )

            # AllReduce across the 8 cores (16 KB)
            cc_in = dp.tile([128, 32], F32, name=f"ci{r}", tag="ci")
            cc_out = dp.tile([128, 32], F32, addr_space="Shared", name=f"co{r}", tag="co")
            nc.sync.dma_start(cc_in[:], bsum_t[:])
            nc.gpsimd.collective_compute(
                "AllReduce",
                mybir.AluOpType.add,
                replica_groups=[list(range(NCORES))],
                ins=[cc_in.opt()],
                outs=[cc_out.opt()],
            )
            msum_t = sp.tile([128, 32], F32, name=f"ms{r}", tag="ms")
            nc.sync.dma_start(msum_t[:], cc_out[:])

            # m_rep = broadcast of msum/(H*W) over CHUNK_L locations
            m_rep = sp.tile([128, OC], F32, name=f"mr{r}", tag="mr")
            nc.scalar.mul(m_rep[:, 0:32], msum_t[:], 1.0 / float(H * W))
            filled = 32
            while filled < OC:
                n = min(filled, OC - filled)
                nc.vector.tensor_copy(
                    out=m_rep[:, filled : filled + n], in_=m_rep[:, 0:n]
                )
                filled += n

            # subtract mean and write out, chunk-wise
            for c in range(NCHUNK):
                oc_t = oc_ts[c]
                nc.vector.tensor_sub(oc_t[:], oc_t[:], m_rep[:])
                nc.sync.dma_start(out_d[:, c * OC : (c + 1) * OC], oc_t[:])

    nc.compile()
    return nc


def _pack_inputs(x, b, weights):
    xs = np.asarray(x, dtype=np.float32).reshape(B, CI, H, W)
    bs = np.asarray(b, dtype=np.float32).reshape(B, CI, H, W)
    ws = np.asarray(weights, dtype=np.float32).reshape(CI, CO, H, W)

    # [CI, H, W, B] activation layouts; moving operand = [s | b] per location
    s_t = np.transpose(xs + bs, (1, 2, 3, 0))
    b_t = np.transpose(bs, (1, 2, 3, 0))
    sb_full = np.concatenate([s_t, b_t], axis=3)  # [128, 48, 48, 64]
    w_t = np.transpose(ws, (0, 2, 3, 1))          # [128, 48, 48, 128]

    in_maps = []
    for c in range(NCORES):
        h0, h1 = c * HL, (c + 1) * HL
        # per chunk (one h-row): w block [128, 6144] then sb block [128, 3072]
        wc = sb_full[:, h0:h1].reshape(128, NCHUNK, SBCOLS)
        ww = w_t[:, h0:h1].reshape(128, NCHUNK, WCOLS)
        dat = np.concatenate([ww, wc], axis=2).reshape(128, NCHUNK * DCOLS)
        in_maps.append({"dat": np.ascontiguousarray(dat)})
    return in_maps


def _unpack_output(res):
    out = np.empty((B, 1, CO, H, W), dtype=np.float32)
    for c in range(NCORES):
        o = res[c]["out"].reshape(128, HL, W, B)  # [j, hl, w, b]
        out[:, 0, :, c * HL : (c + 1) * HL, :] = np.transpose(o, (3, 0, 1, 2))
    return out


def kernel(x: np.ndarray, b: np.ndarray, weights: np.ndarray) -> np.ndarray:
    global LAST_EXEC_TIME_NS

    in_maps = _pack_inputs(x, b, weights)

    if "nc" not in _NC_CACHE:
        _NC_CACHE["nc"] = _build_nc()
    nc = _NC_CACHE["nc"]

    trace = os.environ.get("KERNEL_TRACE", "0") == "1"
    res = run_bass_kernel_spmd(nc, in_maps, list(range(NCORES)), trace=trace)
    LAST_EXEC_TIME_NS = res.exec_time_ns

    return _unpack_output(res.results)
